# revision 8
# baseline (speedup 1.0000x reference)
"""Trainium2 Bass kernel: image -> 2-photon Fock-state basis change.

The reference op is `out[fock_idx] = input_state` with `out` zeros elsewhere
(fock_idx injective), i.e. a pure row scatter [36864, 512] -> [73920, 512].

fock_idx has block structure: input rows [i*192, (i+1)*192) land on output
rows [start(i), start(i)+192) contiguously with start(i) quadratic in i, so
the scatter is 192 contiguous block copies plus zero fills — pure DMA work.

Sharding (fast path): split the *image rows* across the 8 cores — core k
copies blocks 24k..24k+23 with the full 512-wide batch, 384KB per flat
DRAM->DRAM dma_start, 24 instructions per core. The SPMD program stays
uniform by computing each core's output offsets from partition_id in
sequencer registers: local_row(j) = 192 + j*A - j(j-1)/2 with
A = 383 - 24*pid. Each core's output buffer is its slab of the Fock vector
(global rows [start(24k)-192, ...)); the host pastes slabs back together.

Zero rows are never written: the Bass runtime zero-initializes
ExternalOutput buffers (native path pre-zeros; the PJRT path feeds the NEFF
zero-filled buffers). kernel() validates this and repairs + warns if the
contract is ever violated.

A generic batch-sharded path (64 columns per core, one flat dma_start per
contiguous run, no partition_id math) handles any other injective fock_idx.
"""

import numpy as np

D1 = 192
D2 = 192
M = D1 + D2
IMG_DIM = D1 * D2            # 36864
FOCK_DIM = M * (M + 1) // 2  # 73920
BATCH = 512
N_CORES = 8
BS = BATCH // N_CORES        # batch-shard path: 64 columns per core

BPC = D1 // N_CORES          # row-shard path: 24 blocks per core
# uniform per-core output rows: 192 lead margin + largest slab
# (core 7: FOCK_DIM - start(168) = 23412 rows)
OUT_ROWS = 23604


def _fock_indices() -> np.ndarray:
    i = np.repeat(np.arange(D1), D2)
    j = np.tile(np.arange(D2), D1)
    q = D1 + j
    idx = i * M - i * (i - 1) // 2 + (q - i)
    return idx.astype(np.int32)


def _block_starts() -> np.ndarray:
    i = np.arange(D1, dtype=np.int64)
    return i * M - i * (i - 1) // 2 + (D1 - i)


# ---------------------------------------------------------------- planning


def _plan(fock_idx: np.ndarray):
    """Decompose the scatter into contiguous runs + zero intervals."""
    idx = np.asarray(fock_idx, dtype=np.int64).ravel()
    assert idx.shape[0] == IMG_DIM
    assert idx.min() >= 0 and idx.max() < FOCK_DIM
    assert np.unique(idx).size == IMG_DIM, "fock_idx must be injective"

    brk = np.nonzero(np.diff(idx) != 1)[0] + 1
    starts_in = np.concatenate([[0], brk])
    ends_in = np.concatenate([brk, [IMG_DIM]])
    runs = [(int(a), int(idx[a]), int(b - a)) for a, b in zip(starts_in, ends_in)]
    assert len(runs) <= 1024, f"scatter too fragmented: {len(runs)} runs"

    covered = np.zeros(FOCK_DIM, dtype=bool)
    covered[idx] = True
    d = np.diff(covered.astype(np.int8))
    zstarts = np.nonzero(d == -1)[0] + 1
    zends = np.nonzero(d == 1)[0] + 1
    if not covered[0]:
        zstarts = np.concatenate([[0], zstarts])
    if not covered[FOCK_DIM - 1]:
        zends = np.concatenate([zends, [FOCK_DIM]])
    zeros = [(int(a), int(b - a)) for a, b in zip(zstarts, zends)]
    assert sum(r[2] for r in runs) + sum(z[1] for z in zeros) == FOCK_DIM
    return runs, zeros


def _is_fock_pattern(runs) -> bool:
    if len(runs) != D1:
        return False
    starts = _block_starts()
    return all(
        a == i * D2 and ln == D2 and b == int(starts[i])
        for i, (a, b, ln) in enumerate(runs)
    )


# ---------------------------------------------------------------- programs


BLK = D2 * BATCH  # 98304 elems per block copy (384KB)
# Blocks issued in "15-descriptor mode": a 91500-elem DMA (15 descs of
# 6100 elems, engines 0-14) + a 6804-elem residual (14 descs of 486,
# engines 0-13). SDMA engine 15 is ~15-30% slower than its siblings on
# this part (known TRN2 trait) and descriptor k of every DMA goes to
# engine k (restart-at-0, probed), so skipping engine 15 on ~5 of 24
# blocks rebalances the drain so no engine straggles.
MODE15 = (5, 8, 13, 16, 21)
M15_A = 91500  # 15 x 6100
M15_R = BLK - M15_A  # 6804 = 14 x 486


def _build_rowshard_program():
    """Raw bacc kernel (no Tile): 24 block copies spread over the three
    DMA-issuing engines (sync + scalar HWDGE rings, gpsimd SWDGE ring) so
    descriptor generation runs 3-wide and the 16 SDMA engines are fed from
    the start. Tensors are 1-D so sub-row ranges (15-desc mode) are
    expressible. One semaphore wait per engine at the end; the rings
    provide hardware backpressure."""
    import concourse.bacc as bacc
    import concourse.bass as bass
    from concourse import mybir

    nc = bacc.Bacc(
        "TRN2",
        debug=False,
        num_devices=N_CORES,
        enable_asserts=False,
        detect_race_conditions=False,
        monotonic_sem_count=0,
    )
    rows_in = BPC * D2  # 4608
    x = nc.dram_tensor(
        "x", [rows_in * BATCH], mybir.dt.float32, kind="ExternalInput"
    ).ap()
    y = nc.dram_tensor(
        "y", [OUT_ROWS * BATCH], mybir.dt.float32, kind="ExternalOutput"
    ).ap()

    with (
        nc.semaphore("dma_sp") as s_sp,
        nc.semaphore("dma_act") as s_act,
        nc.Block(no_gpsimd_drain=True) as block,
    ):
        half = BLK // 2  # 49152 elems — the pid-free starter slice

        def body(eng, sem, slice_idx, jstart, jstep):
            n = 0
            # block 0 lands at local offset D2*BATCH on every core
            # (pid-free). Split it so both issuing queues have descriptors
            # in flight before the ~1.7us partition_id load.
            e0 = D2 * BATCH + slice_idx * half
            s0 = slice_idx * half
            eng.dma_start(out=y[e0 : e0 + half], in_=x[s0 : s0 + half]).then_inc(
                sem, 16
            )
            n += 1
            pid = eng.partition_id()
            A = eng.snap(383 - pid * BPC)
            # interleave offset computation with DMA issue: the register ops
            # hide inside the ~0.85us per-DMA issue cadence. 15-desc-mode
            # blocks go last so engine 15's backlog ends early.
            late = []
            for j in range(jstart, BPC, jstep):
                tj = j * (j - 1) // 2
                if j in MODE15:
                    late.append((j, tj))
                    continue
                off = eng.snap((A * j + (D2 - tj)) * BATCH)
                eng.dma_start(
                    out=y[bass.ds(off, BLK)],
                    in_=x[j * BLK : (j + 1) * BLK],
                ).then_inc(sem, 16)
                n += 1
            for j, tj in late:
                off = eng.snap((A * j + (D2 - tj)) * BATCH)
                off_r = eng.snap((A * j + (D2 - tj)) * BATCH + M15_A)
                eng.dma_start(
                    out=y[bass.ds(off, M15_A)],
                    in_=x[j * BLK : j * BLK + M15_A],
                ).then_inc(sem, 16)
                eng.dma_start(
                    out=y[bass.ds(off_r, M15_R)],
                    in_=x[j * BLK + M15_A : (j + 1) * BLK],
                ).then_inc(sem, 16)
                n += 2
            eng.wait_ge(sem, 16 * n)

        @block.sync
        def _(sync):
            body(sync, s_sp, 0, 1, 2)

        @block.scalar
        def _(scalar):
            body(scalar, s_act, 1, 2, 2)

    nc.compile()
    return nc


def _build_batchshard_program(runs):
    import concourse.bacc as bacc
    import concourse.tile as tile
    from concourse import mybir

    nc = bacc.Bacc("TRN2", debug=False, num_devices=N_CORES)
    x = nc.dram_tensor("x", [IMG_DIM, BS], mybir.dt.float32, kind="ExternalInput").ap()
    y = nc.dram_tensor(
        "y", [FOCK_DIM, BS], mybir.dt.float32, kind="ExternalOutput"
    ).ap()

    with tile.TileContext(nc) as tc:
        engines = [nc.sync, nc.scalar]
        for k, (a, b, ln) in enumerate(runs):
            engines[k % 2].dma_start(out=y[b : b + ln, :], in_=x[a : a + ln, :])
    nc.compile()
    return nc


_cache = {}


def _get_program(fock_idx: np.ndarray):
    key = hash(np.asarray(fock_idx, dtype=np.int64).tobytes())
    if key not in _cache:
        runs, zeros = _plan(fock_idx)
        if _is_fock_pattern(runs):
            _cache[key] = ("row", _build_rowshard_program(), zeros)
        else:
            _cache[key] = ("batch", _build_batchshard_program(runs), zeros)
    return _cache[key]


# ---------------------------------------------------------------- execution


def _run(nc, in_maps, trace=False, tmpdir=None):
    from concourse import bass_utils

    kw = {"trace": True, "tmpdir": tmpdir} if trace else {}
    return bass_utils.run_bass_kernel_spmd(nc, in_maps, list(range(N_CORES)), **kw)


def _execute(x_full: np.ndarray, fock_idx: np.ndarray, trace=False, tmpdir=None):
    mode, nc, zeros = _get_program(fock_idx)

    if mode == "row":
        rows_in = BPC * D2
        in_maps = [
            {"x": x_full[c * rows_in : (c + 1) * rows_in].reshape(-1)}
            for c in range(N_CORES)
        ]
        res = _run(nc, in_maps, trace, tmpdir)
        starts = _block_starts()
        out = np.zeros((FOCK_DIM, BATCH), dtype=np.float32)
        for k in range(N_CORES):
            g0 = int(starts[BPC * k])
            g1 = int(starts[BPC * (k + 1)]) if k < N_CORES - 1 else FOCK_DIM
            yk = res.results[k]["y"].reshape(OUT_ROWS, BATCH)
            out[g0:g1] = yk[D2 : D2 + (g1 - g0)]
    else:
        in_maps = [
            {"x": np.ascontiguousarray(x_full[:, c * BS : (c + 1) * BS])}
            for c in range(N_CORES)
        ]
        res = _run(nc, in_maps, trace, tmpdir)
        out = np.concatenate([res.results[c]["y"] for c in range(N_CORES)], axis=1)

    # The runtime hands the NEFF zero-initialized output buffers, so
    # unwritten rows must be zero. Validate; repair on the host if the
    # contract is ever violated (should never happen).
    bad = 0
    for r0, length in zeros:
        seg = out[r0 : r0 + length]
        if seg.any():
            bad += int(np.count_nonzero(seg))
            seg[:] = 0
    if bad:
        import sys

        print(
            f"WARNING: output buffer was not zero-initialized "
            f"({bad} nonzero elems in zero rows); repaired on host",
            file=sys.stderr,
        )
    return out, res


def kernel(**inputs) -> np.ndarray:
    x_full = np.ascontiguousarray(np.asarray(inputs["input_state"], dtype=np.float32))
    assert x_full.shape == (IMG_DIM, BATCH)
    fock_idx = inputs.get("fock_idx")
    fock_idx = (
        _fock_indices() if fock_idx is None else np.asarray(fock_idx, dtype=np.int64)
    )
    out, _ = _execute(x_full, fock_idx)
    return out.astype(np.float32, copy=False)

